# revision 4
# baseline (speedup 1.0000x reference)
"""Trainium2 Bass kernel for nn_BlockDiagonalLayer.

Computes out[b, n*64+j] = sin(omega[n] * (sum_i x[b,n,i] * W[n,j,i] + bias[n,j]))
for B=2048, N=1024 networks, D_IN=D_OUT=64, sharded over 8 NeuronCores along N.

Math: with s = (omega/2pi)*(y + b), out = sin(2*pi*s) and only frac(s)
matters.  Host prepares NEGATED scaled weights w'' = -(omega/2pi)*W and bias
rows b'' = -(omega/2pi)*b - 768, so the PSUM accumulates u = -(s + 768) in
(-800, -738).  Then (all exact, hardware-verified):
    k = round(u)            via magic constant M15 = 1.5*2^23
    d = k - u               in [-0.5, 0.5]
    y = sin(2*pi*d) = sin(2*pi*s) = reference output.

Precision (max err 5.8e-3 on the real seed-0 inputs, budget 2e-2):
  x quantized to ~16 bits as xa*2^-8 (fp16, 11-bit int) + xb*2^-9 (fp8e4m3,
  5-bit int residual); the residual matmul uses lhsT fp16 w''*2^-4 against the
  fp8 rhs (mixed-dtype matmul, verified exact on HW).  Weights as fp16 hi+lo
  pair (exact to ~2^-22).  Bias rows as fp16 hi+lo.  Output fp16.

Per core (128 nets = 64 pairs): DMA in groups of PB=4 pairs (2 MB fp16 +
1 MB fp8 in, 2 MB fp16 out; 16KB contiguous per partition) for DMA
efficiency.  Per pair: 16 matmuls (4 kinds x 4 PSUM chunks of 512), range
reduction split 3:1 between ACT-path (Identity(u+M15) on ScalarE +
scalar_tensor_tensor on VectorE) and DVE-path (dual-op tensor_scalar +
tensor_tensor), then Sin on ScalarE (fp16 out).
"""

import numpy as np
import ml_dtypes

import concourse.bass as bass
import concourse.tile as tile
from concourse import bacc, mybir
from concourse.alu_op_type import AluOpType
from concourse.bass_utils import run_bass_kernel_spmd

B, N, D = 2048, 1024, 64
NCORES = 8
NS = N // NCORES          # 128 nets per core
PAIRS = NS // 2           # 64
MMW = 512                 # matmul moving free dim (one PSUM bank)
PB = 4                    # pairs per DMA group

M15 = float(1.5 * 2.0 ** 23)
TWO_PI = float(2.0 * np.pi)

F32 = mybir.dt.float32
F16 = mybir.dt.float16
F8 = mybir.dt.float8e4

FP8 = ml_dtypes.float8_e4m3fn


def build_bass(repeat: int = 1):
    """Build the per-core Bass program (same NEFF on all 8 cores)."""
    nc = bacc.Bacc("TRN2", target_bir_lowering=False, debug=False,
                   num_devices=NCORES)
    xa_d = nc.dram_tensor("xa", [128, PAIRS * B], F16, kind="ExternalInput")
    xb_d = nc.dram_tensor("xb", [128, PAIRS * B], F8, kind="ExternalInput")
    wbd_d = nc.dram_tensor("wbd", [128, PAIRS * 3 * 128], F16,
                           kind="ExternalInput")
    b2_d = nc.dram_tensor("b2", [2, PAIRS * 128], F16, kind="ExternalInput")
    y_d = nc.dram_tensor("y", [128, PAIRS * B], F16, kind="ExternalOutput")

    with tile.TileContext(nc) as tc:
        with (
            tc.tile_pool(name="aux", bufs=1) as aux_pool,
            tc.tile_pool(name="wconst", bufs=1) as wc_pool,
            tc.tile_pool(name="xa", bufs=2) as xa_pool,
            tc.tile_pool(name="xbp", bufs=2) as xb_pool,
            tc.tile_pool(name="tt", bufs=3) as t_pool,
            tc.tile_pool(name="dd", bufs=3) as d_pool,
            tc.tile_pool(name="yy", bufs=2) as y_pool,
            tc.tile_pool(name="ps", bufs=2, space="PSUM") as psum_pool,
        ):
            # --- constants (loaded once, outside the repeat loop) ---
            wsb = wc_pool.tile([128, PAIRS * 3 * 128], F16)
            wchunk = PAIRS * 3 * 128 // 8
            for c in range(8):
                nc.scalar.dma_start(wsb[:, c * wchunk:(c + 1) * wchunk],
                                    wbd_d[:, c * wchunk:(c + 1) * wchunk])
            b2_sb = wc_pool.tile([2, PAIRS * 128], F16)
            nc.gpsimd.dma_start(b2_sb[:], b2_d[:])
            m15_sb = aux_pool.tile([128, 1], F32)
            nc.gpsimd.memset(m15_sb[:], M15)
            ones2 = aux_pool.tile([2, MMW], F16)
            nc.gpsimd.memset(ones2[:], 1.0)

            import contextlib
            rep_ctx = tc.For_i(0, repeat, 1) if repeat > 1 else contextlib.nullcontext()
            with rep_ctx:
                for g in range(PAIRS // PB):
                    c0 = g * PB * B
                    xa = xa_pool.tile([128, PB * B], F16)
                    nc.sync.dma_start(xa[:], xa_d[:, c0:c0 + PB * B])
                    xb = xb_pool.tile([128, PB * B], F8)
                    nc.sync.dma_start(xb[:], xb_d[:, c0:c0 + PB * B])
                    yt = y_pool.tile([128, PB * B], F16)

                    for a in range(PB):
                        p = g * PB + a
                        u = psum_pool.tile([128, B], F32, tag="u")
                        base = p * 3 * 128
                        # kinds 0/1: w16/wlo x xa; kind 2: wr x xb; bias rows
                        for k in range(3):
                            wk = wsb[:, base + k * 128: base + (k + 1) * 128]
                            xop = xa if k < 2 else xb
                            for h in range(B // MMW):
                                nc.tensor.matmul(
                                    u[:, h * MMW:(h + 1) * MMW], wk,
                                    xop[:, a * B + h * MMW: a * B + (h + 1) * MMW],
                                    start=(k == 0), stop=False)
                        bk = b2_sb[:, p * 128:(p + 1) * 128]
                        for h in range(B // MMW):
                            nc.tensor.matmul(
                                u[:, h * MMW:(h + 1) * MMW], bk, ones2[:],
                                start=False, stop=True)

                        d = d_pool.tile([128, B], F32, tag="d")
                        if p % 4 != 3:
                            # ACT path: t = Identity(u + M15) = M15 + round(u)
                            t = t_pool.tile([128, B], F32, tag="t")
                            nc.scalar.activation(
                                t[:], u[:],
                                mybir.ActivationFunctionType.Identity,
                                bias=m15_sb[:], scale=1.0)
                            # d = (t - M15) - u = round(u) - u
                            nc.vector.scalar_tensor_tensor(
                                d[:], t[:], M15, u[:],
                                op0=AluOpType.subtract, op1=AluOpType.subtract)
                        else:
                            # DVE path: k = (u + M15) - M15 = round(u)
                            t = t_pool.tile([128, B], F32, tag="t")
                            nc.vector.tensor_scalar(
                                t[:], u[:], M15, M15,
                                op0=AluOpType.add, op1=AluOpType.subtract)
                            nc.vector.tensor_tensor(
                                d[:], t[:], u[:], op=AluOpType.subtract)

                        nc.scalar.activation(
                            yt[:, a * B:(a + 1) * B], d[:],
                            mybir.ActivationFunctionType.Sin,
                            bias=0.0, scale=TWO_PI)
                    nc.gpsimd.dma_start(y_d[:, c0:c0 + PB * B], yt[:])
    nc.compile()
    return nc


def prep_inputs(x, weights, bias, omega):
    """Host-side quantization + layout prep -> list of 8 per-core dicts."""
    f16 = np.float16
    om = omega.astype(np.float64)
    scl = om / (2.0 * np.pi)

    # --- negated scaled weights, fp16 hi/lo + fp8-path lhsT (fp16) ---
    wp = -(weights.astype(np.float64) * scl[:, None, None])   # [N, 64(j), 64(i)]
    w16 = wp.astype(f16)
    wlo = (wp - w16.astype(np.float64)).astype(f16)
    wr = (wp * 2.0 ** -4).astype(f16)
    # lhsT orientation: [i, j] = wp[n][j, i]
    w16t = w16.transpose(0, 2, 1)
    wlot = wlo.transpose(0, 2, 1)
    wrt = wr.transpose(0, 2, 1)

    bp = -(bias.astype(np.float64) * scl[:, None]) - 768.0    # [N, 64]
    bhi = bp.astype(f16)
    blo = (bp - bhi.astype(np.float64)).astype(f16)

    # fp8 LUT for residual ints in [-16, 16] scaled 2^-9
    lut = (np.arange(-16, 17, dtype=np.float64) * 2.0 ** -9).astype(FP8)

    in_maps = []
    for c in range(NCORES):
        sl = slice(c * NS, (c + 1) * NS)

        # block-diagonal lhsT per (pair, kind): [PAIRS, 3, 128(i2), 128(j2)]
        bd = np.zeros((PAIRS, 3, 128, 128), f16)
        for k, wt in enumerate((w16t, wlot, wrt)):
            wc = wt[sl]                      # [128, 64, 64] (net, i, j)
            bd[:, k, 0:D, 0:D] = wc[0::2]
            bd[:, k, D:, D:] = wc[1::2]
        wbd = np.ascontiguousarray(
            bd.transpose(2, 0, 1, 3).reshape(128, PAIRS * 3 * 128))

        b2 = np.zeros((2, PAIRS * 128), f16)
        bh, bl = bhi[sl], blo[sl]            # [128, 64]
        b2[0] = np.concatenate(
            [bh[0::2], bh[1::2]], axis=1).reshape(-1)
        b2[1] = np.concatenate(
            [bl[0::2], bl[1::2]], axis=1).reshape(-1)

        # x: transpose to [NS, D, B], quantize, then pack [128, PAIRS*B]
        xc = x[:, sl, :]                     # [B, 128, 64]
        xT = np.empty((NS, D, B), np.float32)
        BBLK = 128
        for b0 in range(0, B, BBLK):
            xT[:, :, b0:b0 + BBLK] = xc[b0:b0 + BBLK].transpose(1, 2, 0)
        xi = np.rint(xT.astype(np.float64) * 8192.0).astype(np.int32)
        qa = (xi + 16) >> 5                  # 11-bit ints
        qb = xi - (qa << 5)                  # residual in [-16, 16]
        xa16 = (qa.astype(np.float32) * np.float32(2.0 ** -8)).astype(f16)
        xb8 = lut[(qb + 16).reshape(-1)].reshape(qb.shape)
        # [NS, D, B] -> [PAIRS, 2, D, B] -> [128 (i2), PAIRS*B]
        xa_c = np.ascontiguousarray(
            xa16.reshape(PAIRS, 2, D, B).transpose(1, 2, 0, 3).reshape(
                128, PAIRS * B))
        xb_c = np.ascontiguousarray(
            xb8.reshape(PAIRS, 2, D, B).transpose(1, 2, 0, 3).reshape(
                128, PAIRS * B))

        in_maps.append({"xa": xa_c, "xb": xb_c, "wbd": wbd, "b2": b2})
    return in_maps


def assemble_output(results):
    """[8 cores] of y [128 (j2), PAIRS*B] fp16 -> full [B, N*D] fp32."""
    out = np.empty((B, N * D), np.float32)
    for c in range(NCORES):
        # y[j2, p*B + b] -> rows (p*128 + j2) x cols b
        yy = results[c]["y"].reshape(2, D, PAIRS, B)
        yy = yy.transpose(2, 0, 1, 3).reshape(NS * D, B)
        ov = out[:, c * NS * D:(c + 1) * NS * D]
        for b0 in range(0, B, 128):
            ov[b0:b0 + 128, :] = yy[:, b0:b0 + 128].T.astype(np.float32)
    return out


_NC_CACHE = {}


def kernel(x, weights, bias, omega):
    x = np.ascontiguousarray(x, np.float32)
    weights = np.ascontiguousarray(weights, np.float32)
    bias = np.ascontiguousarray(bias, np.float32)
    omega = np.ascontiguousarray(omega, np.float32)

    if "nc" not in _NC_CACHE:
        _NC_CACHE["nc"] = build_bass()
    nc = _NC_CACHE["nc"]
    in_maps = prep_inputs(x, weights, bias, omega)
    res = run_bass_kernel_spmd(nc, in_maps, core_ids=list(range(NCORES)))
    return assemble_output(res.results)


# revision 5
# speedup vs baseline: 1.1515x; 1.1515x over previous
"""Trainium2 Bass kernel for nn_BlockDiagonalLayer.

Computes out[b, n*64+j] = sin(omega[n] * (sum_i x[b,n,i] * W[n,j,i] + bias[n,j]))
for B=2048, N=1024 networks, D_IN=D_OUT=64, sharded over 8 NeuronCores along N.

Math: with s = (omega/2pi)*(y + b), out = sin(2*pi*s) and only frac(s)
matters.  Host prepares NEGATED scaled weights w'' = -(omega/2pi)*W and bias
rows b'' = -(omega/2pi)*b - 768, so the PSUM accumulates u = -(s + 768) in
(-800, -738).  Then (all exact, hardware-verified):
    k = round(u)            via magic constant M15 = 1.5*2^23
    d = k - u               in [-0.5, 0.5]
    y = sin(2*pi*d) = sin(2*pi*s) = reference output.

Precision (max err 5.8e-3 on the real seed-0 inputs, budget 2e-2):
  x quantized to ~16 bits as xa*2^-8 (fp16, 11-bit int) + xb*2^-9 (fp8e4m3,
  5-bit int residual); the residual matmul uses lhsT fp16 w''*2^-4 against the
  fp8 rhs (mixed-dtype matmul, verified exact on HW).  Weights as fp16 hi+lo
  pair (exact to ~2^-22).  Bias rows as fp16 hi+lo.  Output fp16.

Per core (128 nets = 64 pairs): DMA in groups of PB=4 pairs (2 MB fp16 +
1 MB fp8 in, 2 MB fp16 out; 16KB contiguous per partition) for DMA
efficiency.  Per pair: 16 matmuls (4 kinds x 4 PSUM chunks of 512), range
reduction split 3:1 between ACT-path (Identity(u+M15) on ScalarE +
scalar_tensor_tensor on VectorE) and DVE-path (dual-op tensor_scalar +
tensor_tensor), then Sin on ScalarE (fp16 out).
"""

import numpy as np
import ml_dtypes

import concourse.bass as bass
import concourse.tile as tile
from concourse import bacc, mybir
from concourse.alu_op_type import AluOpType
from concourse.bass_utils import run_bass_kernel_spmd

B, N, D = 2048, 1024, 64
NCORES = 8
NS = N // NCORES          # 128 nets per core
PAIRS = NS // 2           # 64
MMW = 512                 # matmul moving free dim (one PSUM bank)
PB = 2                    # pairs per DMA group

M15 = float(1.5 * 2.0 ** 23)
TWO_PI = float(2.0 * np.pi)

F32 = mybir.dt.float32
F16 = mybir.dt.float16
F8 = mybir.dt.float8e4

FP8 = ml_dtypes.float8_e4m3fn


def build_bass(repeat: int = 1):
    """Build the per-core Bass program (same NEFF on all 8 cores)."""
    nc = bacc.Bacc("TRN2", target_bir_lowering=False, debug=False,
                   num_devices=NCORES)
    xa_d = nc.dram_tensor("xa", [128, PAIRS * B], F16, kind="ExternalInput")
    xb_d = nc.dram_tensor("xb", [128, PAIRS * B], F8, kind="ExternalInput")
    wbd_d = nc.dram_tensor("wbd", [128, PAIRS * 3 * 128], F16,
                           kind="ExternalInput")
    b2_d = nc.dram_tensor("b2", [2, PAIRS * 128], F16, kind="ExternalInput")
    y_d = nc.dram_tensor("y", [128, PAIRS * B], F16, kind="ExternalOutput")

    with tile.TileContext(nc) as tc:
        with (
            tc.tile_pool(name="aux", bufs=1) as aux_pool,
            tc.tile_pool(name="wconst", bufs=1) as wc_pool,
            tc.tile_pool(name="xa", bufs=3) as xa_pool,
            tc.tile_pool(name="xbp", bufs=3) as xb_pool,
            tc.tile_pool(name="tt", bufs=3) as t_pool,
            tc.tile_pool(name="dd", bufs=3) as d_pool,
            tc.tile_pool(name="yy", bufs=3) as y_pool,
            tc.tile_pool(name="ps", bufs=2, space="PSUM") as psum_pool,
        ):
            # --- constants (loaded once, outside the repeat loop) ---
            wsb = wc_pool.tile([128, PAIRS * 3 * 128], F16)
            wchunk = PAIRS * 3 * 128 // 8
            for c in range(8):
                nc.scalar.dma_start(wsb[:, c * wchunk:(c + 1) * wchunk],
                                    wbd_d[:, c * wchunk:(c + 1) * wchunk])
            b2_sb = wc_pool.tile([2, PAIRS * 128], F16)
            nc.gpsimd.dma_start(b2_sb[:], b2_d[:])
            m15_sb = aux_pool.tile([128, 1], F32)
            nc.gpsimd.memset(m15_sb[:], M15)
            ones2 = aux_pool.tile([2, MMW], F16)
            nc.gpsimd.memset(ones2[:], 1.0)

            import contextlib
            rep_ctx = tc.For_i(0, repeat, 1) if repeat > 1 else contextlib.nullcontext()
            with rep_ctx:
                for g in range(PAIRS // PB):
                    c0 = g * PB * B
                    xa = xa_pool.tile([128, PB * B], F16)
                    nc.sync.dma_start(xa[:], xa_d[:, c0:c0 + PB * B])
                    xb = xb_pool.tile([128, PB * B], F8)
                    nc.sync.dma_start(xb[:], xb_d[:, c0:c0 + PB * B])
                    yt = y_pool.tile([128, PB * B], F16)

                    for a in range(PB):
                        p = g * PB + a
                        u = psum_pool.tile([128, B], F32, tag="u")
                        base = p * 3 * 128
                        # kinds 0/1: w16/wlo x xa; kind 2: wr x xb; bias rows
                        for k in range(3):
                            wk = wsb[:, base + k * 128: base + (k + 1) * 128]
                            xop = xa if k < 2 else xb
                            for h in range(B // MMW):
                                nc.tensor.matmul(
                                    u[:, h * MMW:(h + 1) * MMW], wk,
                                    xop[:, a * B + h * MMW: a * B + (h + 1) * MMW],
                                    start=(k == 0), stop=False)
                        bk = b2_sb[:, p * 128:(p + 1) * 128]
                        for h in range(B // MMW):
                            nc.tensor.matmul(
                                u[:, h * MMW:(h + 1) * MMW], bk, ones2[:],
                                start=False, stop=True)

                        d = d_pool.tile([128, B], F32, tag="d")
                        if p % 4 != 3:
                            # ACT path: t = Identity(u + M15) = M15 + round(u)
                            t = t_pool.tile([128, B], F32, tag="t")
                            nc.scalar.activation(
                                t[:], u[:],
                                mybir.ActivationFunctionType.Identity,
                                bias=m15_sb[:], scale=1.0)
                            # d = (t - M15) - u = round(u) - u
                            nc.vector.scalar_tensor_tensor(
                                d[:], t[:], M15, u[:],
                                op0=AluOpType.subtract, op1=AluOpType.subtract)
                        else:
                            # DVE path: k = (u + M15) - M15 = round(u)
                            t = t_pool.tile([128, B], F32, tag="t")
                            nc.vector.tensor_scalar(
                                t[:], u[:], M15, M15,
                                op0=AluOpType.add, op1=AluOpType.subtract)
                            nc.vector.tensor_tensor(
                                d[:], t[:], u[:], op=AluOpType.subtract)

                        nc.scalar.activation(
                            yt[:, a * B:(a + 1) * B], d[:],
                            mybir.ActivationFunctionType.Sin,
                            bias=0.0, scale=TWO_PI)
                    nc.sync.dma_start(y_d[:, c0:c0 + PB * B], yt[:])
    nc.compile()
    return nc


def prep_inputs(x, weights, bias, omega):
    """Host-side quantization + layout prep -> list of 8 per-core dicts."""
    f16 = np.float16
    om = omega.astype(np.float64)
    scl = om / (2.0 * np.pi)

    # --- negated scaled weights, fp16 hi/lo + fp8-path lhsT (fp16) ---
    wp = -(weights.astype(np.float64) * scl[:, None, None])   # [N, 64(j), 64(i)]
    w16 = wp.astype(f16)
    wlo = (wp - w16.astype(np.float64)).astype(f16)
    wr = (wp * 2.0 ** -4).astype(f16)
    # lhsT orientation: [i, j] = wp[n][j, i]
    w16t = w16.transpose(0, 2, 1)
    wlot = wlo.transpose(0, 2, 1)
    wrt = wr.transpose(0, 2, 1)

    bp = -(bias.astype(np.float64) * scl[:, None]) - 768.0    # [N, 64]
    bhi = bp.astype(f16)
    blo = (bp - bhi.astype(np.float64)).astype(f16)

    # fp8 LUT for residual ints in [-16, 16] scaled 2^-9
    lut = (np.arange(-16, 17, dtype=np.float64) * 2.0 ** -9).astype(FP8)

    in_maps = []
    for c in range(NCORES):
        sl = slice(c * NS, (c + 1) * NS)

        # block-diagonal lhsT per (pair, kind): [PAIRS, 3, 128(i2), 128(j2)]
        bd = np.zeros((PAIRS, 3, 128, 128), f16)
        for k, wt in enumerate((w16t, wlot, wrt)):
            wc = wt[sl]                      # [128, 64, 64] (net, i, j)
            bd[:, k, 0:D, 0:D] = wc[0::2]
            bd[:, k, D:, D:] = wc[1::2]
        wbd = np.ascontiguousarray(
            bd.transpose(2, 0, 1, 3).reshape(128, PAIRS * 3 * 128))

        b2 = np.zeros((2, PAIRS * 128), f16)
        bh, bl = bhi[sl], blo[sl]            # [128, 64]
        b2[0] = np.concatenate(
            [bh[0::2], bh[1::2]], axis=1).reshape(-1)
        b2[1] = np.concatenate(
            [bl[0::2], bl[1::2]], axis=1).reshape(-1)

        # x: transpose to [NS, D, B], quantize, then pack [128, PAIRS*B]
        xc = x[:, sl, :]                     # [B, 128, 64]
        xT = np.empty((NS, D, B), np.float32)
        BBLK = 128
        for b0 in range(0, B, BBLK):
            xT[:, :, b0:b0 + BBLK] = xc[b0:b0 + BBLK].transpose(1, 2, 0)
        xi = np.rint(xT.astype(np.float64) * 8192.0).astype(np.int32)
        qa = (xi + 16) >> 5                  # 11-bit ints
        qb = xi - (qa << 5)                  # residual in [-16, 16]
        xa16 = (qa.astype(np.float32) * np.float32(2.0 ** -8)).astype(f16)
        xb8 = lut[(qb + 16).reshape(-1)].reshape(qb.shape)
        # [NS, D, B] -> [PAIRS, 2, D, B] -> [128 (i2), PAIRS*B]
        xa_c = np.ascontiguousarray(
            xa16.reshape(PAIRS, 2, D, B).transpose(1, 2, 0, 3).reshape(
                128, PAIRS * B))
        xb_c = np.ascontiguousarray(
            xb8.reshape(PAIRS, 2, D, B).transpose(1, 2, 0, 3).reshape(
                128, PAIRS * B))

        in_maps.append({"xa": xa_c, "xb": xb_c, "wbd": wbd, "b2": b2})
    return in_maps


def assemble_output(results):
    """[8 cores] of y [128 (j2), PAIRS*B] fp16 -> full [B, N*D] fp32."""
    out = np.empty((B, N * D), np.float32)
    for c in range(NCORES):
        # y[j2, p*B + b] -> rows (p*128 + j2) x cols b
        yy = results[c]["y"].reshape(2, D, PAIRS, B)
        yy = yy.transpose(2, 0, 1, 3).reshape(NS * D, B)
        ov = out[:, c * NS * D:(c + 1) * NS * D]
        for b0 in range(0, B, 128):
            ov[b0:b0 + 128, :] = yy[:, b0:b0 + 128].T.astype(np.float32)
    return out


_NC_CACHE = {}


def kernel(x, weights, bias, omega):
    x = np.ascontiguousarray(x, np.float32)
    weights = np.ascontiguousarray(weights, np.float32)
    bias = np.ascontiguousarray(bias, np.float32)
    omega = np.ascontiguousarray(omega, np.float32)

    if "nc" not in _NC_CACHE:
        _NC_CACHE["nc"] = build_bass()
    nc = _NC_CACHE["nc"]
    in_maps = prep_inputs(x, weights, bias, omega)
    res = run_bass_kernel_spmd(nc, in_maps, core_ids=list(range(NCORES)))
    return assemble_output(res.results)


# revision 6
# speedup vs baseline: 1.4828x; 1.2878x over previous
"""Trainium2 Bass kernel for nn_BlockDiagonalLayer.

Computes out[b, n*64+j] = sin(omega[n] * (sum_i x[b,n,i] * W[n,j,i] + bias[n,j]))
for B=2048, N=1024 networks, D_IN=D_OUT=64, sharded over 8 NeuronCores along N.

Math: with s = (omega/2pi)*(y + b), out = sin(2*pi*s) and only frac(s)
matters.  Host prepares NEGATED scaled weights w'' = -(omega/2pi)*W and bias
rows b'' = -(omega/2pi)*b - 768, so the PSUM accumulates u = -(s + 768) in
(-800, -738).  Then (all exact, hardware-verified):
    k = round(u)            via magic constant M15 = 1.5*2^23
    d = k - u               in [-0.5, 0.5]
    y = sin(2*pi*d) = sin(2*pi*s) = reference output.

Precision (max err 5.8e-3 on the real seed-0 inputs, budget 2e-2):
  x quantized to ~16 bits as xa*2^-8 (fp16, 11-bit int) + xb*2^-9 (fp8e4m3,
  5-bit int residual); the residual matmul uses lhsT fp16 w''*2^-4 against the
  fp8 rhs (mixed-dtype matmul, verified exact on HW).  Weights as fp16 hi+lo
  pair (exact to ~2^-22).  Bias rows as fp16 hi+lo.  Output fp16.

Per core (128 nets = 64 pairs): DMA in groups of PB=4 pairs (2 MB fp16 +
1 MB fp8 in, 2 MB fp16 out; 16KB contiguous per partition) for DMA
efficiency.  Per pair: 16 matmuls (4 kinds x 4 PSUM chunks of 512), range
reduction split 3:1 between ACT-path (Identity(u+M15) on ScalarE +
scalar_tensor_tensor on VectorE) and DVE-path (dual-op tensor_scalar +
tensor_tensor), then Sin on ScalarE (fp16 out).
"""

import numpy as np
import ml_dtypes

import concourse.bass as bass
import concourse.tile as tile
from concourse import bacc, mybir
from concourse.alu_op_type import AluOpType
from concourse.bass_utils import run_bass_kernel_spmd

B, N, D = 2048, 1024, 64
NCORES = 8
NS = N // NCORES          # 128 nets per core
PAIRS = NS // 2           # 64
MMW = 512                 # matmul moving free dim (one PSUM bank)
PB = 2                    # pairs per DMA group
UB = 1024                 # PSUM tile width (2 banks; 4 bufs in flight)

M15 = float(1.5 * 2.0 ** 23)
TWO_PI = float(2.0 * np.pi)

F32 = mybir.dt.float32
F16 = mybir.dt.float16
F8 = mybir.dt.float8e4

FP8 = ml_dtypes.float8_e4m3fn


def build_bass(repeat: int = 1):
    """Build the per-core Bass program (same NEFF on all 8 cores)."""
    nc = bacc.Bacc("TRN2", target_bir_lowering=False, debug=False,
                   num_devices=NCORES)
    xa_d = nc.dram_tensor("xa", [128, PAIRS * B], F16, kind="ExternalInput")
    xb_d = nc.dram_tensor("xb", [128, PAIRS * B], F8, kind="ExternalInput")
    wbd_d = nc.dram_tensor("wbd", [128, PAIRS * 3 * 128], F16,
                           kind="ExternalInput")
    b2_d = nc.dram_tensor("b2", [2, PAIRS * 128], F16, kind="ExternalInput")
    y_d = nc.dram_tensor("y", [128, PAIRS * B], F16, kind="ExternalOutput")

    with tile.TileContext(nc) as tc:
        with (
            tc.tile_pool(name="aux", bufs=1) as aux_pool,
            tc.tile_pool(name="wconst", bufs=1) as wc_pool,
            tc.tile_pool(name="xa", bufs=3) as xa_pool,
            tc.tile_pool(name="xbp", bufs=3) as xb_pool,
            tc.tile_pool(name="tt", bufs=3) as t_pool,
            tc.tile_pool(name="dd", bufs=3) as d_pool,
            tc.tile_pool(name="yy", bufs=3) as y_pool,
            tc.tile_pool(name="ps", bufs=4, space="PSUM") as psum_pool,
        ):
            # --- constants (loaded once, outside the repeat loop) ---
            wsb = wc_pool.tile([128, PAIRS * 3 * 128], F16)
            wchunk = PAIRS * 3 * 128 // 8
            for c in range(8):
                nc.scalar.dma_start(wsb[:, c * wchunk:(c + 1) * wchunk],
                                    wbd_d[:, c * wchunk:(c + 1) * wchunk])
            b2_sb = wc_pool.tile([2, PAIRS * 128], F16)
            nc.gpsimd.dma_start(b2_sb[:], b2_d[:])
            m15_sb = aux_pool.tile([128, 1], F32)
            nc.gpsimd.memset(m15_sb[:], M15)
            ones2 = aux_pool.tile([2, MMW], F16)
            nc.gpsimd.memset(ones2[:], 1.0)

            import contextlib
            rep_ctx = tc.For_i(0, repeat, 1) if repeat > 1 else contextlib.nullcontext()
            with rep_ctx:
                for g in range(PAIRS // PB):
                    c0 = g * PB * B
                    xa = xa_pool.tile([128, PB * B], F16)
                    nc.sync.dma_start(xa[:], xa_d[:, c0:c0 + PB * B])
                    xb = xb_pool.tile([128, PB * B], F8)
                    nc.sync.dma_start(xb[:], xb_d[:, c0:c0 + PB * B])
                    yt = y_pool.tile([128, PB * B], F16)

                    for a in range(PB):
                        p = g * PB + a
                        base = p * 3 * 128
                        for half in range(B // UB):
                            u = psum_pool.tile([128, UB], F32, tag="u")
                            # kinds 0/1: w16/wlo x xa; kind 2: wr x xb; bias
                            for k in range(3):
                                wk = wsb[:, base + k * 128: base + (k + 1) * 128]
                                xop = xa if k < 2 else xb
                                for h in range(UB // MMW):
                                    col = a * B + half * UB + h * MMW
                                    nc.tensor.matmul(
                                        u[:, h * MMW:(h + 1) * MMW], wk,
                                        xop[:, col:col + MMW],
                                        start=(k == 0), stop=False)
                            bk = b2_sb[:, p * 128:(p + 1) * 128]
                            for h in range(UB // MMW):
                                nc.tensor.matmul(
                                    u[:, h * MMW:(h + 1) * MMW], bk, ones2[:],
                                    start=False, stop=True)

                            ycol = a * B + half * UB
                            d = d_pool.tile([128, UB], F32, tag="d")
                            if p % 4 != 3:
                                # ACT path: t = Identity(u+M15) = M15+round(u)
                                t = t_pool.tile([128, UB], F32, tag="t")
                                nc.scalar.activation(
                                    t[:], u[:],
                                    mybir.ActivationFunctionType.Identity,
                                    bias=m15_sb[:], scale=1.0)
                                # d = (t - M15) - u = round(u) - u
                                nc.vector.scalar_tensor_tensor(
                                    d[:], t[:], M15, u[:],
                                    op0=AluOpType.subtract,
                                    op1=AluOpType.subtract)
                            else:
                                # DVE path: k = (u + M15) - M15 = round(u)
                                t = t_pool.tile([128, UB], F32, tag="t")
                                nc.vector.tensor_scalar(
                                    t[:], u[:], M15, M15,
                                    op0=AluOpType.add, op1=AluOpType.subtract)
                                nc.vector.tensor_tensor(
                                    d[:], t[:], u[:], op=AluOpType.subtract)

                            nc.scalar.activation(
                                yt[:, ycol:ycol + UB], d[:],
                                mybir.ActivationFunctionType.Sin,
                                bias=0.0, scale=TWO_PI)
                    nc.sync.dma_start(y_d[:, c0:c0 + PB * B], yt[:])
    nc.compile()
    return nc


def prep_inputs(x, weights, bias, omega):
    """Host-side quantization + layout prep -> list of 8 per-core dicts."""
    f16 = np.float16
    om = omega.astype(np.float64)
    scl = om / (2.0 * np.pi)

    # --- negated scaled weights, fp16 hi/lo + fp8-path lhsT (fp16) ---
    wp = -(weights.astype(np.float64) * scl[:, None, None])   # [N, 64(j), 64(i)]
    w16 = wp.astype(f16)
    wlo = (wp - w16.astype(np.float64)).astype(f16)
    wr = (wp * 2.0 ** -4).astype(f16)
    # lhsT orientation: [i, j] = wp[n][j, i]
    w16t = w16.transpose(0, 2, 1)
    wlot = wlo.transpose(0, 2, 1)
    wrt = wr.transpose(0, 2, 1)

    bp = -(bias.astype(np.float64) * scl[:, None]) - 768.0    # [N, 64]
    bhi = bp.astype(f16)
    blo = (bp - bhi.astype(np.float64)).astype(f16)

    # fp8 LUT for residual ints in [-16, 16] scaled 2^-9
    lut = (np.arange(-16, 17, dtype=np.float64) * 2.0 ** -9).astype(FP8)

    in_maps = []
    for c in range(NCORES):
        sl = slice(c * NS, (c + 1) * NS)

        # block-diagonal lhsT per (pair, kind): [PAIRS, 3, 128(i2), 128(j2)]
        bd = np.zeros((PAIRS, 3, 128, 128), f16)
        for k, wt in enumerate((w16t, wlot, wrt)):
            wc = wt[sl]                      # [128, 64, 64] (net, i, j)
            bd[:, k, 0:D, 0:D] = wc[0::2]
            bd[:, k, D:, D:] = wc[1::2]
        wbd = np.ascontiguousarray(
            bd.transpose(2, 0, 1, 3).reshape(128, PAIRS * 3 * 128))

        b2 = np.zeros((2, PAIRS * 128), f16)
        bh, bl = bhi[sl], blo[sl]            # [128, 64]
        b2[0] = np.concatenate(
            [bh[0::2], bh[1::2]], axis=1).reshape(-1)
        b2[1] = np.concatenate(
            [bl[0::2], bl[1::2]], axis=1).reshape(-1)

        # x: transpose to [NS, D, B], quantize, then pack [128, PAIRS*B]
        xc = x[:, sl, :]                     # [B, 128, 64]
        xT = np.empty((NS, D, B), np.float32)
        BBLK = 128
        for b0 in range(0, B, BBLK):
            xT[:, :, b0:b0 + BBLK] = xc[b0:b0 + BBLK].transpose(1, 2, 0)
        xi = np.rint(xT.astype(np.float64) * 8192.0).astype(np.int32)
        qa = (xi + 16) >> 5                  # 11-bit ints
        qb = xi - (qa << 5)                  # residual in [-16, 16]
        xa16 = (qa.astype(np.float32) * np.float32(2.0 ** -8)).astype(f16)
        xb8 = lut[(qb + 16).reshape(-1)].reshape(qb.shape)
        # [NS, D, B] -> [PAIRS, 2, D, B] -> [128 (i2), PAIRS*B]
        xa_c = np.ascontiguousarray(
            xa16.reshape(PAIRS, 2, D, B).transpose(1, 2, 0, 3).reshape(
                128, PAIRS * B))
        xb_c = np.ascontiguousarray(
            xb8.reshape(PAIRS, 2, D, B).transpose(1, 2, 0, 3).reshape(
                128, PAIRS * B))

        in_maps.append({"xa": xa_c, "xb": xb_c, "wbd": wbd, "b2": b2})
    return in_maps


def assemble_output(results):
    """[8 cores] of y [128 (j2), PAIRS*B] fp16 -> full [B, N*D] fp32."""
    out = np.empty((B, N * D), np.float32)
    for c in range(NCORES):
        # y[j2, p*B + b] -> rows (p*128 + j2) x cols b
        yy = results[c]["y"].reshape(2, D, PAIRS, B)
        yy = yy.transpose(2, 0, 1, 3).reshape(NS * D, B)
        ov = out[:, c * NS * D:(c + 1) * NS * D]
        for b0 in range(0, B, 128):
            ov[b0:b0 + 128, :] = yy[:, b0:b0 + 128].T.astype(np.float32)
    return out


_NC_CACHE = {}


def kernel(x, weights, bias, omega):
    x = np.ascontiguousarray(x, np.float32)
    weights = np.ascontiguousarray(weights, np.float32)
    bias = np.ascontiguousarray(bias, np.float32)
    omega = np.ascontiguousarray(omega, np.float32)

    if "nc" not in _NC_CACHE:
        _NC_CACHE["nc"] = build_bass()
    nc = _NC_CACHE["nc"]
    in_maps = prep_inputs(x, weights, bias, omega)
    res = run_bass_kernel_spmd(nc, in_maps, core_ids=list(range(NCORES)))
    return assemble_output(res.results)
